# revision 17
# baseline (speedup 1.0000x reference)
"""BitLinear-1.58 inference kernel for Trainium2 — hand-scheduled raw bass.

out = (round(x * 128/gamma) @ W^T) * (scale*gamma/128) + bias
with gamma = max(|x|, axis=-1), W ternary {-1,0,1}.

Math (exact): xq = round(x*128/gamma) in [-128, 128] (clip dropped; only
affects +128 vs 127, ~7e-4 global). In the /16 domain xq/16 = xh + r with
xh = fp8e4m3(xq/16) and r = xq/16 - xh: r is a multiple of 1/16 with
|r| <= 1/4, so xh, r, and the ternary weights are all exact in fp8e4m3.
y = 16*((xh + r) @ w) reproduces the integer matmul exactly, while fp8
DoubleRow matmuls contract two 128-chunks per instruction. The 16x folds
into the dequant scale; outputs store as bf16 (~1.2e-3 rel, well inside
tolerance).

Scheduling: no Tile framework — explicit SBUF rings and counting
semaphores, with a fixed software pipeline. Steady-state iteration k:
  PE  : 64 DoubleRow matmuls of tile k          (the 6.8us budget)
  ACT : xh(k+1), xq(k+2), t1(k+3)
  Pool: r(k+1)
  DVE : reduce/scales(k+4), dequant(k)
  SP  : x(k+6) load, transposes(k+2), stores(k-1)
"""

import os
import numpy as np
import ml_dtypes
from contextlib import ExitStack


def _env(k, d):
    return int(os.environ.get(k, d))

import concourse.bass as bass
import concourse.mybir as mybir
from concourse import bacc
from concourse.bass_utils import run_bass_kernel_spmd

N_CORES = 8
B, S, D_IN, D_OUT = 4, 4096, 2048, 2048
TOKENS = B * S
TPC = TOKENS // N_CORES        # 2048 tokens per core
P = 128
N_TILES = TPC // P             # 16
KC = D_IN // P                 # 16
NPAIR = KC // 2                # 8
NF = 512
OC = D_OUT // NF               # 4
MAGIC = 12582912.0             # 1.5*2^23
EPS = 1e-5
Q = 128.0
HD = D_IN // 2

F32 = mybir.dt.float32
BF16 = mybir.dt.bfloat16
F8 = mybir.dt.float8e4
AX = mybir.AxisListType
OP = mybir.AluOpType
AF = mybir.ActivationFunctionType
PM = mybir.MatmulPerfMode

ENG = mybir.EngineType


class Sched:
    """Tick bookkeeping + per-engine wait scoreboard."""

    def __init__(self, nc):
        self.nc = nc
        self.sem = {}
        self.tick = {}
        for name in ("act", "dve", "pool", "pe", "xdma", "tdma", "wdma", "odma"):
            self.sem[name] = nc.alloc_semaphore(f"s_{name}")
            self.tick[name] = 0
        self.seen = {}   # (engine, sem) -> min satisfied value
        self.prod = {}   # logical product -> (sem_name, tick)

    ENG_SEM = {
        mybir.EngineType.Activation: "act",
        mybir.EngineType.DVE: "dve",
        mybir.EngineType.Pool: "pool",
        mybir.EngineType.PE: "pe",
    }

    CHAIN_SET = set(
        os.environ.get("K_CHAIN_ENGS", "act,dve,pool").split(",")
    )

    def emit(self, inst, engine, waits=(), inc=None, inc_by=1, force=()):
        # The engines' wait queues let younger wait-free instructions bypass
        # an older *waiting* one, so data deps within an engine stream still
        # need sems. ACT/DVE/Pool chain strictly on their own semaphore (few
        # large ops, and this makes the foreign-wait scoreboard sound). PE
        # matmuls instead carry `force` waits — per-group data waits that
        # are pre-satisfied in steady state, so they cost latency only.
        own = self.ENG_SEM.get(engine)
        all_waits = list(waits)
        if own in self.CHAIN_SET:
            all_waits.append((own, self.tick[own]))
        attached = set()
        for sem_name, val in force:
            if val <= 0 or (sem_name, val) in attached:
                continue
            attached.add((sem_name, val))
            inst.wait_op(self.sem[sem_name], val, "sem-ge", check=False)
        for sem_name, val in all_waits:
            if val <= 0:
                continue
            key = (engine, sem_name)
            if self.seen.get(key, 0) >= val:
                continue
            self.seen[key] = val
            inst.wait_op(self.sem[sem_name], val, "sem-ge", check=False)
        if inc is None and own is not None:
            inc = own
        if inc is not None:
            self.tick[inc] += inc_by
            inst.then_inc(self.sem[inc], inc_by)
        return inst

    def mark(self, product, sem_name):
        self.prod[product] = (sem_name, self.tick[sem_name])

    def wait_for(self, product):
        if product not in self.prod:
            return None
        return self.prod[product]


def build_kernel(n_tiles=N_TILES):
    nc = bacc.Bacc(
        "TRN2", target_bir_lowering=False, debug=False, num_devices=N_CORES
    )
    tpc = n_tiles * P
    x_d = nc.dram_tensor("x", [tpc, D_IN], F32, kind="ExternalInput").ap()
    w_d = nc.dram_tensor("w", [P, KC * D_OUT], F8, kind="ExternalInput").ap()
    b_d = nc.dram_tensor("bias", [1, D_OUT], BF16, kind="ExternalInput").ap()
    s_d = nc.dram_tensor("scale", [P, 1], F32, kind="ExternalInput").ap()
    o_d = nc.dram_tensor("out", [tpc, D_OUT], BF16, kind="ExternalOutput").ap()

    with ExitStack() as ctx:
        _emit(ctx, nc, o_d, x_d, w_d, b_d, s_d, n_tiles)
    _dedup_ldweights(nc)
    nc.compile()
    return nc


def _dedup_ldweights(nc):
    """Drop InstLdweights whose weights AP matches the previous LDW in the
    same block; merge waits of dropped LDWs into the next kept PE inst."""
    n_removed = 0
    for bb in nc.main_func.blocks:
        kept = []
        last_key = None
        pending_waits = []
        for inst in bb.instructions:
            if isinstance(inst, mybir.InstLdweights):
                key = repr(inst.ins)
                if key == last_key:
                    si = inst.sync_info
                    if si is not None and si.on_wait:
                        pending_waits.extend(si.on_wait)
                    n_removed += 1
                    continue
                last_key = key
            elif isinstance(inst, (mybir.InstMatmult, mybir.InstEventSemaphore)):
                pass
            elif getattr(inst, "engine", None) == ENG.PE:
                last_key = None
            if pending_waits and getattr(inst, "engine", None) == ENG.PE:
                si = inst.sync_info
                if si is None:
                    inst.sync_info = mybir.SyncInfo(
                        on_wait=list(pending_waits), on_update=[]
                    )
                else:
                    si.on_wait = list(si.on_wait) + pending_waits
                pending_waits = []
            kept.append(inst)
        assert not pending_waits
        bb.instructions[:] = kept
    return n_removed


def _emit(ctx, nc, o_d, x_d, w_d, b_d, s_d, n_tiles):
    sc = Sched(nc)

    def sb(name, shape, dtype):
        return ctx.enter_context(nc.sbuf_tensor(name, shape, dtype)).ap()

    # rings
    XR, T1R, QR, QTR, HR, RR, OR = (
        _env("K_XR", 7), _env("K_T1R", 3), _env("K_QR", 3), _env("K_QTR", 3),
        _env("K_HR", 3), _env("K_RR", 3), _env("K_OR", 3),
    )
    x_r = [sb(f"x{i}", [P, D_IN], F32) for i in range(XR)]
    t1_r = [sb(f"t1_{i}", [P, D_IN], F32) for i in range(T1R)]
    xq_r = [sb(f"xq{i}", [P, D_IN], BF16) for i in range(QR)]
    xqT_r = [sb(f"xqT{i}", [P, D_IN], BF16) for i in range(QTR)]
    xh_r = [sb(f"xh{i}", [P, D_IN], F8) for i in range(HR)]
    r_r = [sb(f"r{i}", [P, D_IN], F8) for i in range(RR)]
    o_r = [sb(f"o{i}", [P, D_OUT], BF16) for i in range(OR)]
    w_sb = sb("w_sb", [P, KC * D_OUT], F8)
    bias_sb = sb("bias_sb", [P, D_OUT], F32)
    brow_sb = sb("brow_sb", [1, D_OUT], BF16)
    ones_sb = sb("ones_sb", [1, P], BF16)
    magic_sb = sb("magic_sb", [P, 1], F32)
    nmagic_sb = sb("nmagic_sb", [P, 1], F32)
    zero_sb = sb("zero_sb", [P, 1], F32)
    scale_sb = sb("scale_sb", [P, 1], F32)
    warm_sb = sb("warm_sb", [P, 1], F32)
    SM = 8
    ga_r = sb("ga_r", [P, SM], F32)
    gb_r = sb("gb_r", [P, SM], F32)
    g2_r = sb("g2_r", [P, SM], F32)
    g2b_r = sb("g2b_r", [P, SM], F32)
    inv_r = sb("inv_r", [P, SM], F32)
    deq_r = sb("deq_r", [P, SM], F32)

    ps_r = [
        ctx.enter_context(nc.psum_tensor(f"ps{i}", [P, D_OUT], F32)).ap()
        for i in range(2)
    ]

    w3 = w_sb.rearrange("p (c o) -> p c o", c=KC)

    # ---------- constants / warmup ----------
    sc.emit(nc.vector.memset(magic_sb[:], MAGIC), ENG.DVE, [], inc="dve")
    sc.emit(nc.vector.memset(nmagic_sb[:], -MAGIC / 16.0), ENG.DVE, [], inc="dve")
    sc.emit(nc.vector.memset(zero_sb[:], 0.0), ENG.DVE, [], inc="dve")
    sc.mark("consts", "dve")
    sc.emit(nc.gpsimd.memset(ones_sb[:], 1.0), ENG.Pool, [], inc="pool")
    sc.mark("ones", "pool")
    # touch ScalarE so the activation table load happens during DMA fill;
    # the wait also orders every later ACT op after the const memsets
    sc.emit(
        nc.scalar.activation(warm_sb[:], magic_sb[:], AF.Identity,
                             bias=magic_sb[:, 0:1]),
        ENG.Activation, waits=[sc.wait_for("consts")], inc="act",
    )

    # ---------- emission helpers ----------
    def load_x(k, split=False):
        slot = x_r[k % XR]
        # WAR: overwrites x(k-XR), last read by t1(k-XR)
        w = []
        pw = sc.wait_for(f"t1:{k - XR}")
        if pw:
            w.append(pw)
        if split:
            i1 = nc.sync.dma_start(slot[:, :HD], x_d[k * P:(k + 1) * P, :HD])
            sc.emit(i1, ENG.SP, [(n, v) for n, v in w], inc="xdma", inc_by=16)
            sc.mark(f"xa:{k}", "xdma")
            i2 = nc.sync.dma_start(slot[:, HD:], x_d[k * P:(k + 1) * P, HD:])
            sc.emit(i2, ENG.SP, [], inc="xdma", inc_by=16)
        else:
            i1 = nc.sync.dma_start(slot[:], x_d[k * P:(k + 1) * P, :])
            sc.emit(i1, ENG.SP, [(n, v) for n, v in w], inc="xdma", inc_by=16)
        sc.mark(f"x:{k}", "xdma")

    def load_w_pair(cp):
        i = nc.sync.dma_start(
            w_sb[:, 2 * cp * D_OUT:(2 * cp + 2) * D_OUT],
            w_d[:, 2 * cp * D_OUT:(2 * cp + 2) * D_OUT],
        )
        sc.emit(i, ENG.SP, [], inc="wdma", inc_by=16)
        sc.mark(f"w:{cp}", "wdma")

    def dve_front(k):
        m = k % SM
        xa = x_r[k % XR]
        w = [sc.wait_for(f"xa:{k}") or sc.wait_for(f"x:{k}")]
        i = nc.vector.tensor_reduce(
            ga_r[:, m:m + 1], xa[:, :HD], axis=AX.X, op=OP.max,
            apply_absolute_value=True,
        )
        sc.emit(i, ENG.DVE, [x for x in w if x], inc="dve")
        w = [sc.wait_for(f"x:{k}")]
        i = nc.vector.tensor_reduce(
            gb_r[:, m:m + 1], xa[:, HD:], axis=AX.X, op=OP.max,
            apply_absolute_value=True,
        )
        sc.emit(i, ENG.DVE, [x for x in w if x], inc="dve")
        i = nc.vector.scalar_tensor_tensor(
            g2_r[:, m:m + 1], ga_r[:, m:m + 1], EPS, gb_r[:, m:m + 1],
            OP.max, OP.max,
        )
        sc.emit(i, ENG.DVE, [], inc="dve")
        i = nc.vector.tensor_scalar(
            g2b_r[:, m:m + 1], g2_r[:, m:m + 1], 1.0 / Q, None, OP.mult
        )
        sc.emit(i, ENG.DVE, [], inc="dve")
        i = nc.vector.reciprocal(inv_r[:, m:m + 1], g2b_r[:, m:m + 1])
        sc.emit(i, ENG.DVE, [], inc="dve")
        sc.mark(f"inv:{k}", "dve")
        i = nc.vector.tensor_scalar(
            deq_r[:, m:m + 1], g2b_r[:, m:m + 1], scale_sb[:, 0:1], None, OP.mult
        )
        w2 = [sc.wait_for("scale")]
        sc.emit(i, ENG.DVE, [x for x in w2 if x], inc="dve")
        sc.mark(f"deq:{k}", "dve")

    def act_t1(k):
        m = k % SM
        # WAR: t1(k) overwrites t1(k-T1R), whose half-b is read by DVE's
        # dve_xq(k-T1R, 1) — without this wait a lagging DVE races ACT
        w = [sc.wait_for(f"x:{k}"), sc.wait_for(f"inv:{k}"),
             sc.wait_for(f"xq1:{k - T1R}")]
        i = nc.scalar.activation(
            t1_r[k % T1R][:], x_r[k % XR][:], AF.Identity,
            bias=magic_sb[:, 0:1], scale=inv_r[:, m:m + 1],
        )
        sc.emit(i, ENG.Activation, [x for x in w if x], inc="act")
        sc.mark(f"t1:{k}", "act")

    def act_xq(k, half):
        # xq16 = (t1 - MAGIC)/16 = xq/16, exact in bf16 (same significand)
        # WAR: xq(k) overwrites xq(k-QR), read by transposes(k-QR)
        sl = slice(half * HD, (half + 1) * HD)
        w = [sc.wait_for(f"tp{half}:{k - QR}")]
        i = nc.scalar.activation(
            xq_r[k % QR][:, sl], t1_r[k % T1R][:, sl], AF.Identity,
            bias=nmagic_sb[:, 0:1], scale=1.0 / 16.0,
        )
        sc.emit(i, ENG.Activation, [x for x in w if x], inc="act")
        sc.mark(f"xq{half}:{k}", "act")

    def act_xh(k, half):
        # xh = fp8 cast of xq16T, in halves so r and the matmuls start early
        # WAR: xh(k) overwrites xh(k-HR); its last reader is mm(k-HR)
        sl = slice(half * HD, (half + 1) * HD)
        w = [sc.wait_for(f"tp{half}:{k}"), sc.wait_for(f"mm:{k - HR}")]
        i = nc.scalar.activation(
            xh_r[k % HR][:, sl], xqT_r[k % QTR][:, sl], AF.Identity,
            bias=zero_sb[:, 0:1], scale=1.0,
        )
        sc.emit(i, ENG.Activation, [x for x in w if x], inc="act")
        sc.mark(f"xh{half}:{k}", "act")

    def dve_xq(k, half):
        # xq16 on DVE (engine balance): (t1 * 1/16) + (-MAGIC/16), bf16 out
        sl = slice(half * HD, (half + 1) * HD)
        w = [sc.wait_for(f"tp{half}:{k - QR}"), sc.wait_for(f"t1:{k}")]
        i = nc.vector.tensor_scalar(
            xq_r[k % QR][:, sl], t1_r[k % T1R][:, sl],
            1.0 / 16.0, -MAGIC / 16.0, OP.mult, OP.add,
        )
        sc.emit(i, ENG.DVE, [x for x in w if x], inc="dve")
        sc.mark(f"xq{half}:{k}", "dve")

    def pool_r(k, half):
        # r = xq16T - xh ; fp8 out. WAR: overwrites r(k-RR) read by mm(k-RR)
        sl = slice(half * HD, (half + 1) * HD)
        w = [sc.wait_for(f"xh{half}:{k}"), sc.wait_for(f"mm:{k - RR}")]
        i = nc.gpsimd.tensor_tensor(
            r_r[k % RR][:, sl], xqT_r[k % QTR][:, sl], xh_r[k % HR][:, sl],
            OP.subtract,
        )
        sc.emit(i, ENG.Pool, [x for x in w if x], inc="pool")
        sc.mark(f"r{half}:{k}", "pool")

    def sp_transpose(k, half):
        xqT3 = xqT_r[k % QTR].rearrange("p (c t) -> p c t", c=KC)
        xq = xq_r[k % QR]
        hk = KC // 2
        sl = slice(half * HD, (half + 1) * HD)
        cs = slice(half * hk, (half + 1) * hk)
        # WAR: overwrites xqT(k-QTR), read by xh/r(k-QTR)
        w = [sc.wait_for(f"xq{half}:{k}"), sc.wait_for(f"r1:{k - QTR}"),
             sc.wait_for(f"xh1:{k - QTR}")]
        i = nc.sync.dma_start_transpose(xqT3[:, cs, :], xq[:, sl])
        sc.emit(i, ENG.SP, [x for x in w if x], inc="tdma", inc_by=16)
        sc.mark(f"tp{half}:{k}", "tdma")

    def pe_mm(k):
        ps = ps_r[k % 2]
        xh3 = xh_r[k % HR].rearrange("p (c t) -> p c t", c=KC)
        r3 = r_r[k % RR].rearrange("p (c t) -> p c t", c=KC)
        # every matmul carries its data waits explicitly (pre-satisfied in
        # steady state): the wait-queue bypass otherwise lets younger
        # wait-free matmuls run before a blocked group's inputs exist
        wait_xh = [sc.wait_for(f"xh0:{k}"), sc.wait_for(f"xh1:{k}")]
        wait_r = [sc.wait_for(f"r0:{k}"), sc.wait_for(f"r1:{k}")]
        wait_ps = sc.wait_for(f"deq2:{k - 2}")
        wait_bias = sc.wait_for("bias_cp")
        for c in range(NPAIR):
            for h, sT3 in ((0, xh3), (1, r3)):
                for oc in range(OC):
                    half = 0 if c < NPAIR // 2 else 1
                    f = [wait_xh[half] if h == 0 else wait_r[half]]
                    if c == 0 and h == 0:
                        f.append(wait_ps)
                        f.append(wait_bias)
                    if k <= 2:
                        f.append(sc.wait_for(f"w:{c}"))
                    i = nc.tensor.matmul(
                        ps[:, oc * NF:(oc + 1) * NF],
                        sT3[:, 2 * c:2 * c + 2, :],
                        w3[:, 2 * c:2 * c + 2, oc * NF:(oc + 1) * NF],
                        start=(c == 0 and h == 0),
                        stop=(c == NPAIR - 1 and h == 1),
                        perf_mode=PM.DoubleRow,
                    )
                    sc.emit(i, ENG.PE, [], force=[x for x in f if x])
        sc.mark(f"mm:{k}", "pe")

    def dve_deq(k, nch=2):
        ps = ps_r[k % 2]
        o_t = o_r[k % OR]
        m = k % SM
        cw = D_OUT // nch
        for hc in range(nch):
            sl = slice(hc * cw, (hc + 1) * cw)
            w = [sc.wait_for(f"mm:{k}")]
            if hc == 0:
                w.append(sc.wait_for(f"stlast:{k - OR}"))
            i = nc.vector.scalar_tensor_tensor(
                o_t[:, sl], ps[:, sl], deq_r[:, m:m + 1], bias_sb[:, sl],
                OP.mult, OP.add,
            )
            sc.emit(i, ENG.DVE, [x for x in w if x], inc="dve")
            sc.mark(f"deq{hc + 1}:{k}", "dve")

    def sp_store(k, nch=2):
        o_t = o_r[k % OR]
        cw = D_OUT // nch
        for hc in range(nch):
            sl = slice(hc * cw, (hc + 1) * cw)
            w = [sc.wait_for(f"deq{hc + 1}:{k}")]
            i = nc.sync.dma_start(o_d[k * P:(k + 1) * P, sl], o_t[:, sl])
            sc.emit(i, ENG.SP, [x for x in w if x], inc="odma", inc_by=16)
            sc.mark(f"st{hc + 1}:{k}", "odma")
        sc.mark(f"stlast:{k}", "odma")

    # ---------- bias broadcast (K=1 matmul against ones) ----------
    def bias_broadcast_brow():
        i = nc.sync.dma_start(brow_sb[:], b_d[:])
        sc.emit(i, ENG.SP, [], inc="wdma", inc_by=16)
        sc.mark("brow", "wdma")

    def bias_broadcast_rest():
        psB = ps_r[0]
        for oc in range(OC):
            on = slice(oc * NF, (oc + 1) * NF)
            i = nc.tensor.matmul(psB[:, on], ones_sb[:], brow_sb[:, on],
                                 start=True, stop=True)
            w = [sc.wait_for("brow"), sc.wait_for("ones")]
            sc.emit(i, ENG.PE, [], force=[x for x in w if x],
                    inc="pe" if oc == OC - 1 else None)
        sc.mark("bias_mm", "pe")
        i = nc.scalar.copy(bias_sb[:], psB[:])
        sc.emit(i, ENG.Activation, [sc.wait_for("bias_mm")], inc="act")
        sc.mark("bias_cp", "act")
        # mm(0) overwrites psB: wait bias_cp (attached at pe_mm first)

    # ---------- scale ----------
    i = nc.sync.dma_start(scale_sb[:], s_d[:])
    sc.emit(i, ENG.SP, [], inc="wdma", inc_by=16)
    sc.mark("scale", "wdma")

    # ---------- prologue ----------
    # SP/DMA order: x0, x1, tp(0), w0..w3, tp(1), x2, w4..w7, x3, x4 —
    # tile-0/1 criticals first, weights streamed behind them (the matmuls
    # of tiles 0..2 carry per-pair w waits)
    bias_broadcast_brow()
    load_x(0, split=True)
    bias_broadcast_rest()   # ACT copy sits before t1(0); PE mms run early
    load_x(1)
    dve_front(0)
    act_t1(0)
    act_xq(0, 0)
    sp_transpose(0, 0)
    act_xq(0, 1)
    sp_transpose(0, 1)
    dve_front(1)
    act_t1(1)
    for cp in range(0, 4):
        load_w_pair(cp)
    act_xh(0, 0)
    pool_r(0, 0)
    act_xq(1, 0)
    sp_transpose(1, 0)
    act_xh(0, 1)
    pool_r(0, 1)
    act_xq(1, 1)
    sp_transpose(1, 1)
    load_x(2)
    for cp in range(4, NPAIR):
        load_w_pair(cp)
    dve_front(2)
    act_t1(2)
    load_x(3)
    dve_front(3)
    load_x(4)

    # ---------- steady state ----------
    # ACT order per iteration: xh_a(k+1), xq_a(k+2), xh_b(k+1), xq_b(k+2),
    # t1(k+3) — per-half chains keep the transpose loop off the critical path
    for k in range(n_tiles):
        if k + 5 < n_tiles:
            load_x(k + 5)
        if k + 1 < n_tiles:
            act_xh(k + 1, 0)
            pool_r(k + 1, 0)
        if k + 2 < n_tiles:
            act_xq(k + 2, 0)
            sp_transpose(k + 2, 0)
            dve_xq(k + 2, 1)
        if k + 1 < n_tiles:
            act_xh(k + 1, 1)
            pool_r(k + 1, 1)
        if k + 2 < n_tiles:
            sp_transpose(k + 2, 1)
        if k + 3 < n_tiles:
            act_t1(k + 3)
        if k + 4 < n_tiles:
            dve_front(k + 4)
        pe_mm(k)
        dve_deq(k)
        if k - 1 >= 0:
            sp_store(k - 1)
    sp_store(n_tiles - 1)


def prep_inputs(x, quantized_weight, scale, bias):
    x = np.asarray(x, dtype=np.float32)
    quantized_weight = np.asarray(quantized_weight, dtype=np.float32)
    scale = np.asarray(scale, dtype=np.float32)
    bias = np.asarray(bias, dtype=np.float32)
    xf = np.ascontiguousarray(x.reshape(-1, D_IN))
    wT = quantized_weight.T.astype(ml_dtypes.float8_e4m3fn)
    w_prep = np.ascontiguousarray(
        wT.reshape(KC, P, D_OUT).transpose(1, 0, 2).reshape(P, KC * D_OUT)
    )
    bias_bc = np.ascontiguousarray(
        bias.reshape(1, D_OUT).astype(ml_dtypes.bfloat16)
    )
    scale_bc = np.full((P, 1), np.float32(scale) * 16.0, dtype=np.float32)
    return xf, w_prep, bias_bc, scale_bc


_NC_CACHE = {}


def get_nc(n_tiles=N_TILES):
    if n_tiles not in _NC_CACHE:
        _NC_CACHE[n_tiles] = build_kernel(n_tiles)
    return _NC_CACHE[n_tiles]


def kernel(x, quantized_weight, scale, bias, _trace=False):
    xf, w_prep, bias_bc, scale_bc = prep_inputs(x, quantized_weight, scale, bias)
    in_maps = [
        {
            "x": xf[i * TPC:(i + 1) * TPC],
            "w": w_prep,
            "bias": bias_bc,
            "scale": scale_bc,
        }
        for i in range(N_CORES)
    ]
    nc = get_nc()
    res = run_bass_kernel_spmd(nc, in_maps, list(range(N_CORES)), trace=_trace)
    out = np.concatenate(
        [np.asarray(res.results[i]["out"]) for i in range(N_CORES)], axis=0
    )
    out = out.astype(np.float32).reshape(B, S, D_OUT)
    if _trace:
        return out, res
    return out
